# revision 4
# baseline (speedup 1.0000x reference)
"""MoE (16 routed experts, top-4 sigmoid gating, + shared expert) on 8 TRN2 cores.

Strategy: expert-parallel. Core c owns routed experts {2c, 2c+1} and a
64-column slice of the shared expert's intermediate dimension.

Per core (SPMD, identical program, per-core data):
  - gate: scores = sigmoid(x @ gate_w.T) computed in full fp32 for all 2048
    tokens (gate_w columns are permuted per-core so this core's experts are
    always columns 0 and 1 of the score matrix).
  - top-4 of 16 via 4x (reduce_max + mask); normalized weights for the two
    owned experts.
  - routed experts: dense SwiGLU over all tokens in float32r (fp32 with
    11-bit mantissa; 4x faster PE throughput), output scaled per-token by the
    combine weight (zero for tokens not routed here) and summed with the
    shared-expert I-slice partial.
  - 4 chunked ReduceScatters (one per 512-token block) combine partials
    across cores; each core ends with 4x64 token rows, reassembled on host.
"""
import sys

for _p in ("/opt/trn_rl_repo", "/root/.axon_site/_ro/pypackages"):
    if _p not in sys.path:
        sys.path.insert(0, _p)

import numpy as np
from concourse import bacc, tile, mybir
from concourse import bass_utils

dt = mybir.dt
AF = mybir.ActivationFunctionType
ALU = mybir.AluOpType

B, S, H, I, E, TOPK = 2, 1024, 1024, 512, 16, 4
T = B * S                  # 2048 tokens
NCORES = 8
EPC = E // NCORES          # 2 experts per core
ISH = I // NCORES          # 64 shared-intermediate columns per core
P = 128
HC = H // P                # 8 contraction chunks
NTB = 4                    # token blocks
TBS = T // NTB             # 512 tokens per block
ITILES = I // P            # 4 intermediate tiles per expert
NEG = -1.0e9

_CACHE = {}


def _round_f32r(a: np.ndarray) -> np.ndarray:
    """RNE-round fp32 to f32r (11 explicit mantissa bits; low 12 bits zero)."""
    u = np.ascontiguousarray(a, dtype=np.float32).view(np.uint32)
    lsb = (u >> np.uint32(12)) & np.uint32(1)
    r = (u + np.uint32(0x7FF) + lsb) & np.uint32(0xFFFFF000)
    return r.view(np.float32)


def _build():
    nc = bacc.Bacc("TRN2", target_bir_lowering=False, debug=False,
                   num_devices=NCORES)
    f32, f32r = dt.float32, dt.float32r

    xT = nc.dram_tensor("xT", [H, T], f32r, kind="ExternalInput").ap()
    xTf = nc.dram_tensor("xTf", [H, T], f32, kind="ExternalInput").ap()
    gwT = nc.dram_tensor("gwT", [H, E], f32, kind="ExternalInput").ap()
    wg = nc.dram_tensor("wg", [EPC, H, I], f32r, kind="ExternalInput").ap()
    wu = nc.dram_tensor("wu", [EPC, H, I], f32r, kind="ExternalInput").ap()
    wd = nc.dram_tensor("wd", [EPC, I, H], f32r, kind="ExternalInput").ap()
    sg = nc.dram_tensor("sg", [H, ISH], f32r, kind="ExternalInput").ap()
    su = nc.dram_tensor("su", [H, ISH], f32r, kind="ExternalInput").ap()
    sd = nc.dram_tensor("sd", [ISH, H], f32r, kind="ExternalInput").ap()
    id16 = nc.dram_tensor("id16", [16, 16], f32, kind="ExternalInput").ap()
    out = nc.dram_tensor("out", [NTB * (TBS // NCORES), H], f32,
                         kind="ExternalOutput").ap()

    with tile.TileContext(nc) as tc:
        from contextlib import ExitStack
        with ExitStack() as ctx:
            wp = ctx.enter_context(tc.tile_pool(name="wp", bufs=1))
            xqp = ctx.enter_context(tc.tile_pool(name="xqp", bufs=2))
            xfp = ctx.enter_context(tc.tile_pool(name="xfp", bufs=3))
            scp = ctx.enter_context(tc.tile_pool(name="scp", bufs=4))
            tmp = ctx.enter_context(tc.tile_pool(name="tmp", bufs=8))
            ap_ = ctx.enter_context(tc.tile_pool(name="ap", bufs=1))
            op_ = ctx.enter_context(tc.tile_pool(name="op", bufs=2))
            ps1 = ctx.enter_context(tc.tile_pool(name="ps1", bufs=4, space="PSUM"))
            ps2 = ctx.enter_context(tc.tile_pool(name="ps2", bufs=3, space="PSUM"))
            dram = ctx.enter_context(tc.tile_pool(name="dram", bufs=1, space="DRAM"))

            # ---- resident weights ----
            wg_sb = [[wp.tile([P, I], f32r, tag=f"wg{e}_{h}", name=f"wg{e}_{h}") for h in range(HC)]
                     for e in range(EPC)]
            wu_sb = [[wp.tile([P, I], f32r, tag=f"wu{e}_{h}", name=f"wu{e}_{h}") for h in range(HC)]
                     for e in range(EPC)]
            wd_sb = [[wp.tile([P, H], f32r, tag=f"wd{e}_{i}", name=f"wd{e}_{i}") for i in range(ITILES)]
                     for e in range(EPC)]
            sg_sb = [wp.tile([P, ISH], f32r, tag=f"sg{h}", name=f"sg{h}") for h in range(HC)]
            su_sb = [wp.tile([P, ISH], f32r, tag=f"su{h}", name=f"su{h}") for h in range(HC)]
            sd_sb = wp.tile([ISH, H], f32r, tag="sd")
            gw_sb = [wp.tile([P, E], f32, tag=f"gw{h}", name=f"gw{h}") for h in range(HC)]
            id_sb = wp.tile([16, 16], f32, tag="id16")
            w_sb = wp.tile([P, 2 * (T // P)], f32, tag="wsb")  # combine weights

            for e in range(EPC):
                for h in range(HC):
                    nc.sync.dma_start(out=wg_sb[e][h][:], in_=wg[e, h * P:(h + 1) * P, :])
                    nc.sync.dma_start(out=wu_sb[e][h][:], in_=wu[e, h * P:(h + 1) * P, :])
                for i in range(ITILES):
                    nc.sync.dma_start(out=wd_sb[e][i][:], in_=wd[e, i * P:(i + 1) * P, :])
            for h in range(HC):
                nc.sync.dma_start(out=sg_sb[h][:], in_=sg[h * P:(h + 1) * P, :])
                nc.sync.dma_start(out=su_sb[h][:], in_=su[h * P:(h + 1) * P, :])
                nc.sync.dma_start(out=gw_sb[h][:], in_=gwT[h * P:(h + 1) * P, :])
            nc.sync.dma_start(out=sd_sb[:], in_=sd)
            nc.sync.dma_start(out=id_sb[:], in_=id16)

            # ---- gate: fp32 scores + sigmoid + transpose + top-4 weights ----
            for tc4 in range(T // 512):
                pg = ps1.tile([16, 512], f32, tag="ps1")
                for h in range(HC):
                    xf = xfp.tile([P, 512], f32, tag="xf")
                    nc.sync.dma_start(
                        out=xf[:], in_=xTf[h * P:(h + 1) * P, tc4 * 512:(tc4 + 1) * 512])
                    nc.tensor.matmul(pg[:], lhsT=gw_sb[h][:], rhs=xf[:],
                                     start=(h == 0), stop=(h == HC - 1))
                scs = scp.tile([16, 512], f32, tag="scs")
                nc.scalar.activation(scs[:], pg[:], AF.Sigmoid)
                for j in range(4):
                    tt = tc4 * 4 + j
                    pt = ps2.tile([P, 16], f32, tag="ps2")
                    nc.tensor.transpose(pt[:], scs[:, j * P:(j + 1) * P], id_sb[:])
                    s = scp.tile([P, 16], f32, tag="sc")
                    nc.scalar.copy(s[:], pt[:])
                    # top-4 via 4x (max + mask-out)
                    ms = []
                    cur = s
                    for k in range(4):
                        mk = tmp.tile([P, 1], f32, tag="m1")
                        nc.vector.reduce_max(mk[:], cur[:], axis=mybir.AxisListType.X)
                        ms.append(mk)
                        if k < 3:
                            bk = tmp.tile([P, 16], f32, tag="b16")
                            nc.vector.tensor_scalar(bk[:], cur[:], mk[:], None, op0=ALU.is_ge)
                            nxt = tmp.tile([P, 16], f32, tag="s16")
                            nc.vector.scalar_tensor_tensor(
                                nxt[:], bk[:], NEG, cur[:], op0=ALU.mult, op1=ALU.add)
                            cur = nxt
                    d1 = tmp.tile([P, 1], f32, tag="m1")
                    nc.vector.tensor_tensor(d1[:], ms[0][:], ms[1][:], ALU.add)
                    d2 = tmp.tile([P, 1], f32, tag="m1")
                    nc.vector.tensor_tensor(d2[:], ms[2][:], ms[3][:], ALU.add)
                    den = tmp.tile([P, 1], f32, tag="m1")
                    nc.vector.tensor_tensor(den[:], d1[:], d2[:], ALU.add)
                    rden = tmp.tile([P, 1], f32, tag="m1")
                    nc.vector.reciprocal(rden[:], den[:])
                    for e in range(EPC):
                        be = tmp.tile([P, 1], f32, tag="m1")
                        nc.vector.tensor_scalar(be[:], s[:, e:e + 1], ms[3][:], None,
                                                op0=ALU.is_ge)
                        num = tmp.tile([P, 1], f32, tag="m1")
                        nc.vector.tensor_tensor(num[:], s[:, e:e + 1], be[:], ALU.mult)
                        nc.vector.tensor_tensor(w_sb[:, 2 * tt + e:2 * tt + e + 1],
                                                num[:], rden[:], ALU.mult)

            # ---- experts + shared, block by block; chunked ReduceScatter ----
            rs_outs = []
            for tb in range(NTB):
                t0 = tb * TBS
                xq = [xqp.tile([P, TBS], f32r, tag=f"xq{h}", name=f"xq{tb}_{h}") for h in range(HC)]
                for h in range(HC):
                    nc.sync.dma_start(out=xq[h][:],
                                      in_=xT[h * P:(h + 1) * P, t0:t0 + TBS])

                # stage 1: aT[e] = silu(Wg_e.T x) * (Wu_e.T x), f32r  [I, TBS]
                aT = [[ap_.tile([P, TBS], f32r, tag=f"a{e}_{i}", name=f"a{tb}_{e}_{i}") for i in range(ITILES)]
                      for e in range(EPC)]
                for e in range(EPC):
                    for it in range(ITILES):
                        pgu = ps1.tile([P, TBS], f32, tag="ps1")
                        puu = ps1.tile([P, TBS], f32, tag="ps1")
                        for h in range(HC):
                            nc.tensor.matmul(
                                pgu[:], lhsT=wg_sb[e][h][:, it * P:(it + 1) * P],
                                rhs=xq[h][:], start=(h == 0), stop=(h == HC - 1))
                            nc.tensor.matmul(
                                puu[:], lhsT=wu_sb[e][h][:, it * P:(it + 1) * P],
                                rhs=xq[h][:], start=(h == 0), stop=(h == HC - 1))
                        sil = tmp.tile([P, TBS], f32, tag="sil", bufs=3)
                        nc.scalar.activation(sil[:], pgu[:], AF.Silu)
                        nc.vector.tensor_tensor(aT[e][it][:], sil[:], puu[:], ALU.mult)

                # shared expert I-slice
                psg = ps1.tile([ISH, TBS], f32, tag="ps1")
                psu = ps1.tile([ISH, TBS], f32, tag="ps1")
                for h in range(HC):
                    nc.tensor.matmul(psg[:], lhsT=sg_sb[h][:], rhs=xq[h][:],
                                     start=(h == 0), stop=(h == HC - 1))
                    nc.tensor.matmul(psu[:], lhsT=su_sb[h][:], rhs=xq[h][:],
                                     start=(h == 0), stop=(h == HC - 1))
                ssil = tmp.tile([ISH, TBS], f32, tag="ssil", bufs=2)
                nc.scalar.activation(ssil[:], psg[:], AF.Silu)
                ash = ap_.tile([ISH, TBS], f32r, tag="ash")
                nc.vector.tensor_tensor(ash[:], ssil[:], psu[:], ALU.mult)

                # stage 2: partial[t, :] = sh + w0*eo0 + w1*eo1  -> bounce
                bounce = dram.tile([TBS, H], f32, tag=f"bounce{tb}")
                for j in range(TBS // P):
                    tt = tb * (TBS // P) + j
                    for hh in range(H // 512):
                        psh = ps2.tile([P, 512], f32, tag="ps2")
                        nc.tensor.matmul(
                            psh[:], lhsT=ash[:, j * P:(j + 1) * P],
                            rhs=sd_sb[:, hh * 512:(hh + 1) * 512],
                            start=True, stop=True)
                        pe0 = ps2.tile([P, 512], f32, tag="ps2")
                        pe1 = ps2.tile([P, 512], f32, tag="ps2")
                        for e, pe in ((0, pe0), (1, pe1)):
                            for ic in range(ITILES):
                                nc.tensor.matmul(
                                    pe[:], lhsT=aT[e][ic][:, j * P:(j + 1) * P],
                                    rhs=wd_sb[e][ic][:, hh * 512:(hh + 1) * 512],
                                    start=(ic == 0), stop=(ic == ITILES - 1))
                        o0 = op_.tile([P, 512], f32, tag="o0")
                        nc.scalar.copy(o0[:], psh[:])
                        o1 = op_.tile([P, 512], f32, tag="o1")
                        nc.vector.scalar_tensor_tensor(
                            o1[:], pe0[:], w_sb[:, 2 * tt:2 * tt + 1], o0[:],
                            op0=ALU.mult, op1=ALU.add)
                        o2 = op_.tile([P, 512], f32, tag="o2")
                        nc.vector.scalar_tensor_tensor(
                            o2[:], pe1[:], w_sb[:, 2 * tt + 1:2 * tt + 2], o1[:],
                            op0=ALU.mult, op1=ALU.add)
                        nc.sync.dma_start(
                            out=bounce[j * P:(j + 1) * P, hh * 512:(hh + 1) * 512],
                            in_=o2[:])

                rso = dram.tile([TBS // NCORES, H], f32, tag=f"rso{tb}")
                nc.gpsimd.collective_compute(
                    "ReduceScatter", ALU.add,
                    ins=[bounce[:].opt()], outs=[rso[:].opt()],
                    replica_groups=[list(range(NCORES))])
                rs_outs.append(rso)

            rows = TBS // NCORES  # 64
            for tb, rso in enumerate(rs_outs):
                nc.sync.dma_start(out=out[tb * rows:(tb + 1) * rows, :], in_=rso[:])

    nc.compile()
    return nc


def kernel(hidden_states, gate_w, Wg, Wu, Wd, sg, su, sd):
    if "nc" not in _CACHE:
        _CACHE["nc"] = _build()
    nc = _CACHE["nc"]

    x = np.ascontiguousarray(np.asarray(hidden_states, dtype=np.float32)).reshape(T, H)
    gate_w = np.asarray(gate_w, dtype=np.float32)
    Wg = np.asarray(Wg, dtype=np.float32)
    Wu = np.asarray(Wu, dtype=np.float32)
    Wd = np.asarray(Wd, dtype=np.float32)
    sg = np.asarray(sg, dtype=np.float32)
    su = np.asarray(su, dtype=np.float32)
    sd = np.asarray(sd, dtype=np.float32)

    xT_f = np.ascontiguousarray(x.T)
    xT_r = _round_f32r(xT_f)
    id16 = np.eye(16, dtype=np.float32)

    in_maps = []
    for c in range(NCORES):
        mine = [2 * c, 2 * c + 1]
        perm = mine + [e for e in range(E) if e not in mine]
        in_maps.append({
            "xT": xT_r,
            "xTf": xT_f,
            "gwT": np.ascontiguousarray(gate_w[perm].T),
            "wg": _round_f32r(Wg[mine]),
            "wu": _round_f32r(Wu[mine]),
            "wd": _round_f32r(Wd[mine]),
            "sg": _round_f32r(sg[:, c * ISH:(c + 1) * ISH]),
            "su": _round_f32r(su[:, c * ISH:(c + 1) * ISH]),
            "sd": _round_f32r(sd[c * ISH:(c + 1) * ISH, :]),
            "id16": id16,
        })

    res = bass_utils.run_bass_kernel_spmd(nc, in_maps, core_ids=list(range(NCORES)))

    # Reassemble: block tb's ReduceScatter hands core c global token rows
    # [tb*TBS + c*rows, tb*TBS + (c+1)*rows).
    rows = TBS // NCORES
    full = np.empty((T, H), dtype=np.float32)
    for c in range(NCORES):
        oc = res.results[c]["out"]
        for tb in range(NTB):
            g0 = tb * TBS + c * rows
            full[g0:g0 + rows] = oc[tb * rows:(tb + 1) * rows]
    return full.reshape(B, S, H)


# revision 5
# speedup vs baseline: 1145.9849x; 1145.9849x over previous
"""MoE (16 routed experts, top-4 sigmoid gating, + shared expert) on 8 TRN2 cores.

Strategy: expert-parallel. Core c owns routed experts {2c, 2c+1} and a
64-column slice of the shared expert's intermediate dimension.

Per core (SPMD, identical program, per-core data):
  - gate: scores = sigmoid(x @ gate_w.T) computed in full fp32 for all 2048
    tokens (gate_w columns are permuted per-core so this core's experts are
    always columns 0 and 1 of the score matrix).
  - top-4 of 16 via 4x (reduce_max + mask); normalized weights for the two
    owned experts.
  - routed experts: dense SwiGLU over all tokens in float32r (fp32 with
    11-bit mantissa; 4x faster PE throughput), output scaled per-token by the
    combine weight (zero for tokens not routed here) and summed with the
    shared-expert I-slice partial.
  - 4 chunked ReduceScatters (one per 512-token block) combine partials
    across cores; each core ends with 4x64 token rows, reassembled on host.
"""
import sys

for _p in ("/opt/trn_rl_repo", "/root/.axon_site/_ro/pypackages"):
    if _p not in sys.path:
        sys.path.insert(0, _p)

import numpy as np
from concourse import bacc, tile, mybir
from concourse import bass_utils

dt = mybir.dt
AF = mybir.ActivationFunctionType
ALU = mybir.AluOpType

B, S, H, I, E, TOPK = 2, 1024, 1024, 512, 16, 4
T = B * S                  # 2048 tokens
NCORES = 8
EPC = E // NCORES          # 2 experts per core
ISH = I // NCORES          # 64 shared-intermediate columns per core
P = 128
HC = H // P                # 8 contraction chunks
NTB = 4                    # token blocks
TBS = T // NTB             # 512 tokens per block
ITILES = I // P            # 4 intermediate tiles per expert
NEG = -1.0e9

_CACHE = {}


def _round_f32r(a: np.ndarray) -> np.ndarray:
    """RNE-round fp32 to f32r (11 explicit mantissa bits; low 12 bits zero)."""
    u = np.ascontiguousarray(a, dtype=np.float32).view(np.uint32)
    lsb = (u >> np.uint32(12)) & np.uint32(1)
    r = (u + np.uint32(0x7FF) + lsb) & np.uint32(0xFFFFF000)
    return r.view(np.float32)


def _build():
    nc = bacc.Bacc("TRN2", target_bir_lowering=False, debug=False,
                   num_devices=NCORES)
    f32, f32r = dt.float32, dt.float32r

    xT = nc.dram_tensor("xT", [H, T], f32r, kind="ExternalInput").ap()
    xTf = nc.dram_tensor("xTf", [H, T], f32, kind="ExternalInput").ap()
    gwT = nc.dram_tensor("gwT", [H, E], f32, kind="ExternalInput").ap()
    wg = nc.dram_tensor("wg", [EPC, H, I], f32r, kind="ExternalInput").ap()
    wu = nc.dram_tensor("wu", [EPC, H, I], f32r, kind="ExternalInput").ap()
    wd = nc.dram_tensor("wd", [EPC, I, H], f32r, kind="ExternalInput").ap()
    sg = nc.dram_tensor("sg", [H, ISH], f32r, kind="ExternalInput").ap()
    su = nc.dram_tensor("su", [H, ISH], f32r, kind="ExternalInput").ap()
    sd = nc.dram_tensor("sd", [ISH, H], f32r, kind="ExternalInput").ap()
    id16 = nc.dram_tensor("id16", [16, 16], f32, kind="ExternalInput").ap()
    out = nc.dram_tensor("out", [NTB * (TBS // NCORES), H], f32,
                         kind="ExternalOutput").ap()

    with tile.TileContext(nc) as tc:
        from contextlib import ExitStack
        with ExitStack() as ctx:
            wp = ctx.enter_context(tc.tile_pool(name="wp", bufs=1))
            xqp = ctx.enter_context(tc.tile_pool(name="xqp", bufs=2))
            xfp = ctx.enter_context(tc.tile_pool(name="xfp", bufs=3))
            scp = ctx.enter_context(tc.tile_pool(name="scp", bufs=4))
            tmp = ctx.enter_context(tc.tile_pool(name="tmp", bufs=8))
            ap_ = ctx.enter_context(tc.tile_pool(name="ap", bufs=1))
            op_ = ctx.enter_context(tc.tile_pool(name="op", bufs=2))
            ps1 = ctx.enter_context(tc.tile_pool(name="ps1", bufs=4, space="PSUM"))
            ps2 = ctx.enter_context(tc.tile_pool(name="ps2", bufs=3, space="PSUM"))
            dram = ctx.enter_context(tc.tile_pool(name="dram", bufs=1, space="DRAM"))

            # ---- resident weights ----
            wg_sb = [[wp.tile([P, I], f32r, tag=f"wg{e}_{h}", name=f"wg{e}_{h}") for h in range(HC)]
                     for e in range(EPC)]
            wu_sb = [[wp.tile([P, I], f32r, tag=f"wu{e}_{h}", name=f"wu{e}_{h}") for h in range(HC)]
                     for e in range(EPC)]
            wd_sb = [[wp.tile([P, H], f32r, tag=f"wd{e}_{i}", name=f"wd{e}_{i}") for i in range(ITILES)]
                     for e in range(EPC)]
            sg_sb = [wp.tile([P, ISH], f32r, tag=f"sg{h}", name=f"sg{h}") for h in range(HC)]
            su_sb = [wp.tile([P, ISH], f32r, tag=f"su{h}", name=f"su{h}") for h in range(HC)]
            sd_sb = wp.tile([ISH, H], f32r, tag="sd")
            gw_sb = [wp.tile([P, E], f32, tag=f"gw{h}", name=f"gw{h}") for h in range(HC)]
            id_sb = wp.tile([16, 16], f32, tag="id16")
            w_sb = wp.tile([P, 2 * (T // P)], f32, tag="wsb")  # combine weights

            for e in range(EPC):
                for h in range(HC):
                    nc.sync.dma_start(out=wg_sb[e][h][:], in_=wg[e, h * P:(h + 1) * P, :])
                    nc.sync.dma_start(out=wu_sb[e][h][:], in_=wu[e, h * P:(h + 1) * P, :])
                for i in range(ITILES):
                    nc.sync.dma_start(out=wd_sb[e][i][:], in_=wd[e, i * P:(i + 1) * P, :])
            for h in range(HC):
                nc.sync.dma_start(out=sg_sb[h][:], in_=sg[h * P:(h + 1) * P, :])
                nc.sync.dma_start(out=su_sb[h][:], in_=su[h * P:(h + 1) * P, :])
                nc.sync.dma_start(out=gw_sb[h][:], in_=gwT[h * P:(h + 1) * P, :])
            nc.sync.dma_start(out=sd_sb[:], in_=sd)
            nc.sync.dma_start(out=id_sb[:], in_=id16)

            # ---- gate: fp32 scores + sigmoid + transpose + top-4 weights ----
            for tc4 in range(T // 512):
                pg = ps1.tile([16, 512], f32, tag="ps1")
                for h in range(HC):
                    xf = xfp.tile([P, 512], f32, tag="xf")
                    nc.sync.dma_start(
                        out=xf[:], in_=xTf[h * P:(h + 1) * P, tc4 * 512:(tc4 + 1) * 512])
                    nc.tensor.matmul(pg[:], lhsT=gw_sb[h][:], rhs=xf[:],
                                     start=(h == 0), stop=(h == HC - 1))
                scs = scp.tile([16, 512], f32, tag="scs")
                nc.scalar.activation(scs[:], pg[:], AF.Sigmoid)
                for j in range(4):
                    tt = tc4 * 4 + j
                    pt = ps2.tile([P, 16], f32, tag="ps2")
                    nc.tensor.transpose(pt[:], scs[:, j * P:(j + 1) * P], id_sb[:])
                    s = scp.tile([P, 16], f32, tag="sc")
                    nc.scalar.copy(s[:], pt[:])
                    # top-4 via 4x (max + mask-out)
                    ms = []
                    cur = s
                    for k in range(4):
                        mk = tmp.tile([P, 1], f32, tag="m1")
                        nc.vector.reduce_max(mk[:], cur[:], axis=mybir.AxisListType.X)
                        ms.append(mk)
                        if k < 3:
                            bk = tmp.tile([P, 16], f32, tag="b16")
                            nc.vector.tensor_scalar(bk[:], cur[:], mk[:], None, op0=ALU.is_ge)
                            nxt = tmp.tile([P, 16], f32, tag="s16")
                            nc.vector.scalar_tensor_tensor(
                                nxt[:], bk[:], NEG, cur[:], op0=ALU.mult, op1=ALU.add)
                            cur = nxt
                    d1 = tmp.tile([P, 1], f32, tag="m1")
                    nc.vector.tensor_tensor(d1[:], ms[0][:], ms[1][:], ALU.add)
                    d2 = tmp.tile([P, 1], f32, tag="m1")
                    nc.vector.tensor_tensor(d2[:], ms[2][:], ms[3][:], ALU.add)
                    den = tmp.tile([P, 1], f32, tag="m1")
                    nc.vector.tensor_tensor(den[:], d1[:], d2[:], ALU.add)
                    rden = tmp.tile([P, 1], f32, tag="m1")
                    nc.vector.reciprocal(rden[:], den[:])
                    for e in range(EPC):
                        be = tmp.tile([P, 1], f32, tag="m1")
                        nc.vector.tensor_scalar(be[:], s[:, e:e + 1], ms[3][:], None,
                                                op0=ALU.is_ge)
                        num = tmp.tile([P, 1], f32, tag="m1")
                        nc.vector.tensor_tensor(num[:], s[:, e:e + 1], be[:], ALU.mult)
                        nc.vector.tensor_tensor(w_sb[:, 2 * tt + e:2 * tt + e + 1],
                                                num[:], rden[:], ALU.mult)

            # ---- experts + shared, block by block; chunked ReduceScatter ----
            rs_outs = []
            for tb in range(NTB):
                t0 = tb * TBS
                xq = [xqp.tile([P, TBS], f32r, tag=f"xq{h}", name=f"xq{tb}_{h}") for h in range(HC)]
                for h in range(HC):
                    nc.sync.dma_start(out=xq[h][:],
                                      in_=xT[h * P:(h + 1) * P, t0:t0 + TBS])

                # stage 1: aT[e] = silu(Wg_e.T x) * (Wu_e.T x), f32r  [I, TBS]
                aT = [[ap_.tile([P, TBS], f32r, tag=f"a{e}_{i}", name=f"a{tb}_{e}_{i}") for i in range(ITILES)]
                      for e in range(EPC)]
                for e in range(EPC):
                    for it in range(ITILES):
                        pgu = ps1.tile([P, TBS], f32, tag="ps1")
                        puu = ps1.tile([P, TBS], f32, tag="ps1")
                        for h in range(HC):
                            nc.tensor.matmul(
                                pgu[:], lhsT=wg_sb[e][h][:, it * P:(it + 1) * P],
                                rhs=xq[h][:], start=(h == 0), stop=(h == HC - 1))
                            nc.tensor.matmul(
                                puu[:], lhsT=wu_sb[e][h][:, it * P:(it + 1) * P],
                                rhs=xq[h][:], start=(h == 0), stop=(h == HC - 1))
                        sil = tmp.tile([P, TBS], f32, tag="sil", bufs=3)
                        nc.scalar.activation(sil[:], pgu[:], AF.Silu)
                        nc.vector.tensor_tensor(aT[e][it][:], sil[:], puu[:], ALU.mult)

                # shared expert I-slice
                psg = ps1.tile([ISH, TBS], f32, tag="ps1")
                psu = ps1.tile([ISH, TBS], f32, tag="ps1")
                for h in range(HC):
                    nc.tensor.matmul(psg[:], lhsT=sg_sb[h][:], rhs=xq[h][:],
                                     start=(h == 0), stop=(h == HC - 1))
                    nc.tensor.matmul(psu[:], lhsT=su_sb[h][:], rhs=xq[h][:],
                                     start=(h == 0), stop=(h == HC - 1))
                ssil = tmp.tile([ISH, TBS], f32, tag="ssil", bufs=2)
                nc.scalar.activation(ssil[:], psg[:], AF.Silu)
                ash = ap_.tile([ISH, TBS], f32r, tag="ash")
                nc.vector.tensor_tensor(ash[:], ssil[:], psu[:], ALU.mult)

                # stage 2: partial[t, :] = sh + w0*eo0 + w1*eo1  -> bounce
                bounce = dram.tile([TBS, H], f32, tag=f"bounce{tb}")
                for j in range(TBS // P):
                    tt = tb * (TBS // P) + j
                    for hh in range(H // 512):
                        psh = ps2.tile([P, 512], f32, tag="ps2")
                        nc.tensor.matmul(
                            psh[:], lhsT=ash[:, j * P:(j + 1) * P],
                            rhs=sd_sb[:, hh * 512:(hh + 1) * 512],
                            start=True, stop=True)
                        pe0 = ps2.tile([P, 512], f32, tag="ps2")
                        pe1 = ps2.tile([P, 512], f32, tag="ps2")
                        for e, pe in ((0, pe0), (1, pe1)):
                            for ic in range(ITILES):
                                nc.tensor.matmul(
                                    pe[:], lhsT=aT[e][ic][:, j * P:(j + 1) * P],
                                    rhs=wd_sb[e][ic][:, hh * 512:(hh + 1) * 512],
                                    start=(ic == 0), stop=(ic == ITILES - 1))
                        o0 = op_.tile([P, 512], f32, tag="o0")
                        nc.scalar.copy(o0[:], psh[:])
                        o1 = op_.tile([P, 512], f32, tag="o1")
                        nc.vector.scalar_tensor_tensor(
                            o1[:], pe0[:], w_sb[:, 2 * tt:2 * tt + 1], o0[:],
                            op0=ALU.mult, op1=ALU.add)
                        o2 = op_.tile([P, 512], f32, tag="o2")
                        nc.vector.scalar_tensor_tensor(
                            o2[:], pe1[:], w_sb[:, 2 * tt + 1:2 * tt + 2], o1[:],
                            op0=ALU.mult, op1=ALU.add)
                        nc.sync.dma_start(
                            out=bounce[j * P:(j + 1) * P, hh * 512:(hh + 1) * 512],
                            in_=o2[:])

                rso = dram.tile([TBS // NCORES, H], f32, tag=f"rso{tb}")
                nc.gpsimd.collective_compute(
                    "ReduceScatter", ALU.add,
                    ins=[bounce[:].opt()], outs=[rso[:].opt()],
                    replica_groups=[list(range(NCORES))])
                rs_outs.append(rso)

            rows = TBS // NCORES  # 64
            for tb, rso in enumerate(rs_outs):
                nc.sync.dma_start(out=out[tb * rows:(tb + 1) * rows, :], in_=rso[:])

    nc.compile()
    return nc


def kernel(hidden_states, gate_w, Wg, Wu, Wd, sg, su, sd):
    if "nc" not in _CACHE:
        _CACHE["nc"] = _build()
    nc = _CACHE["nc"]

    x = np.ascontiguousarray(np.asarray(hidden_states, dtype=np.float32)).reshape(T, H)
    gate_w = np.asarray(gate_w, dtype=np.float32)
    Wg = np.asarray(Wg, dtype=np.float32)
    Wu = np.asarray(Wu, dtype=np.float32)
    Wd = np.asarray(Wd, dtype=np.float32)
    sg = np.asarray(sg, dtype=np.float32)
    su = np.asarray(su, dtype=np.float32)
    sd = np.asarray(sd, dtype=np.float32)

    xT_f = np.ascontiguousarray(x.T)
    xT_r = _round_f32r(xT_f)
    id16 = np.eye(16, dtype=np.float32)

    in_maps = []
    for c in range(NCORES):
        mine = [2 * c, 2 * c + 1]
        perm = mine + [e for e in range(E) if e not in mine]
        in_maps.append({
            "xT": xT_r,
            "xTf": xT_f,
            "gwT": np.ascontiguousarray(gate_w[perm].T),
            "wg": _round_f32r(Wg[mine]),
            "wu": _round_f32r(Wu[mine]),
            "wd": _round_f32r(Wd[mine]),
            "sg": _round_f32r(sg[:, c * ISH:(c + 1) * ISH]),
            "su": _round_f32r(su[:, c * ISH:(c + 1) * ISH]),
            "sd": _round_f32r(sd[c * ISH:(c + 1) * ISH, :]),
            "id16": id16,
        })

    _CACHE["in_maps"] = in_maps
    res = bass_utils.run_bass_kernel_spmd(nc, in_maps, core_ids=list(range(NCORES)))

    # Reassemble: block tb's ReduceScatter hands core c global token rows
    # [tb*TBS + c*rows, tb*TBS + (c+1)*rows).
    rows = TBS // NCORES
    full = np.empty((T, H), dtype=np.float32)
    for c in range(NCORES):
        oc = res.results[c]["out"]
        for tb in range(NTB):
            g0 = tb * TBS + c * rows
            full[g0:g0 + rows] = oc[tb * rows:(tb + 1) * rows]
    return full.reshape(B, S, H)
